# revision 1
# baseline (speedup 1.0000x reference)
"""Trainium2 Bass kernel for a 2-head MultiHeadAttn + residual + LayerNorm block.

Problem shapes (hardcoded):
  x:      [8, 2048, 384] f32      attn_mask: [8, 2048] bool (True = attend)
  qkv_w:  [384, 384] f32          qkv_b: [384] f32
  o_w:    [128, 384] f32          ln_g, ln_b: [384] f32
  out:    [8, 2048, 384] f32

Sharding: data-parallel over batch — 8 batch elements, one per NeuronCore.
Each core runs the identical program (SPMD) on its own batch slice.

Per-core dataflow (everything stays on-chip; S=2048, D_model=384, H=2, Dh=64):
  1. load x [2048,384] -> SBUF tiled [128, 16, 384]
  2. PE-transpose x -> xT [128, 3, 2048]  (model dim on partitions)
  3. qkvT = (x @ qkv_w).T computed directly as [128 j, 2048 s] via
     lhsT=qkv_w chunk, rhs=xT  (j = qkv dim; j-tile 0/1/2 = Q^T/K^T/V^T,
     partitions of each = 2 heads x 64 head dims)
  4. per head h: scores^T tile [128 k, q] = lhsT(K^T chunk).T @ rhs(Q^T)
     exp(scale*s) fused in ONE scalar-engine pass PSUM->SBUF (no max
     subtraction: |scores*scale| < ~8 for this distribution, exp is safe)
  5. pv: lhsT = Vtil [128 k-chunk, 65] (V rows scaled by mask + a mask/ones
     column), rhs = expS^T -> psum [65, q] accumulating over k chunks:
     rows 0..63 = unnormalized attn^T, row 64 = softmax denominator.
     Masking is exact: masked k rows of Vtil are zeroed so they drop out of
     both numerator and denominator.
  6. denominators: DMA psum row -> stage [2, 2048], PE-transpose to [128, 2*16],
     reciprocal.
  7. per head o-projection psum_h [128 s, 384]; combine with fused DVE ops:
     y = (psum_h0 * r0 + x) ; y = (psum_h1 * r1 + y)   (r = 1/denom per row)
  8. LayerNorm over 384 via bn_stats/bn_aggr + sqrt(var+eps) + reciprocal,
     out = (y - mean) * rstd [* g + b], DMA out.
"""

import os
import sys

import ml_dtypes
import numpy as np

for _p in ("/opt/trn_rl_repo", "/root/.axon_site/_ro/trn_rl_repo"):
    if os.path.isdir(_p) and _p not in sys.path:
        sys.path.insert(0, _p)

import concourse.bass as bass  # noqa: E402
import concourse.tile as tile  # noqa: E402
from concourse import bacc  # noqa: E402
from concourse import mybir  # noqa: E402
from concourse.bass_utils import run_bass_kernel_spmd  # noqa: E402
from concourse.masks import make_identity  # noqa: E402

FP = mybir.dt.float32
BF = mybir.dt.bfloat16
AF = mybir.ActivationFunctionType
OP = mybir.AluOpType

B, S, DM = 8, 2048, 384
H, DH = 2, 64
INNER = H * DH  # 128
P = 128
SC = S // P  # 16 s-chunks of 128
DC = DM // P  # 3 model-dim chunks of 128
NQ = S // 512  # 4 q-tiles of 512
LN_EPS = 1e-3
N_CORES = 8
SCALE = 1.0 / (DH**0.5)


def _build(
    has_mask: bool,
    has_bias: bool,
    has_affine: bool,
    reps: int = 1,
    phases: str = "pre,att,post",
) -> bass.Bass:
    ph = set(phases.split(","))
    # Bacc (not raw Bass): its compile() pipeline legalizes semaphore waits
    # (TRN2 allows at most one sync wait per instruction) via
    # move_matmul_waits_to_ldweights + generate_event_semaphores.
    nc = bacc.Bacc(
        "TRN2", target_bir_lowering=False, debug=False, num_devices=N_CORES
    )

    x_d = nc.dram_tensor("x", [S, DM], FP, kind="ExternalInput")
    xb_d = nc.dram_tensor("x_bf", [S, DM], BF, kind="ExternalInput")
    w_d = nc.dram_tensor("qkv_w_bf", [DM, 3 * INNER], BF, kind="ExternalInput")
    ow_d = nc.dram_tensor("o_w_bf", [INNER, DM], BF, kind="ExternalInput")
    mask_d = bias_d = g_d = b_d = None
    if has_mask:
        mask_d = nc.dram_tensor("mask_f", [S], FP, kind="ExternalInput")
    if has_bias:
        bias_d = nc.dram_tensor("qkv_b", [3 * INNER], FP, kind="ExternalInput")
    if has_affine:
        g_d = nc.dram_tensor("ln_g", [DM], FP, kind="ExternalInput")
        b_d = nc.dram_tensor("ln_b", [DM], FP, kind="ExternalInput")
    y_d = nc.dram_tensor("y", [S, DM], FP, kind="ExternalOutput")

    with tile.TileContext(nc) as tc:
        with tc.tile_pool(name="singles", bufs=1) as sg:
            ident = sg.tile([P, P], FP, tag="ident")
            make_identity(nc, ident)

            x_sb = sg.tile([P, SC, DM], FP, tag="x_sb")

            w_sb = sg.tile([P, DC, 3 * INNER], BF, tag="w_sb")
            nc.sync.dma_start(w_sb, w_d.rearrange("(dc dp) j -> dp dc j", dp=P))
            # o_w split per head with head-dim on partitions 0..63 so each
            # head's matmul operands share base partition 0
            ow_sb = sg.tile([DH, H, DM], BF, tag="ow_sb")
            nc.sync.dma_start(ow_sb, ow_d.rearrange("(h d) m -> d h m", d=DH))

            eps_sb = sg.tile([P, 1], FP, tag="eps")
            nc.vector.memset(eps_sb, LN_EPS)

            mask_sb = bias_sb = g_sb = b_sb = None
            if mask_d is not None:
                mask_sb = sg.tile([P, SC], FP, tag="mask_sb")
                nc.sync.dma_start(mask_sb, mask_d.rearrange("(c p) -> p c", p=P))
            if bias_d is not None:
                bias_sb = sg.tile([P, 3], FP, tag="bias_sb")
                nc.sync.dma_start(bias_sb, bias_d.rearrange("(jt p) -> p jt", p=P))
            if g_d is not None and b_d is not None:
                g_sb = sg.tile([P, DM], FP, tag="g_sb")
                b_sb = sg.tile([P, DM], FP, tag="b_sb")
                nc.gpsimd.dma_start(g_sb, g_d[None, :].to_broadcast((P, DM)))
                nc.gpsimd.dma_start(b_sb, b_d[None, :].to_broadcast((P, DM)))

            rep_ctx = (
                tc.For_i(
                    0,
                    reps,
                    1,
                    hint_engines=(
                        mybir.EngineType.PE,
                        mybir.EngineType.DVE,
                        mybir.EngineType.Activation,
                        mybir.EngineType.SP,
                    ),
                )
                if reps > 1
                else None
            )
            if rep_ctx is not None:
                rep_ctx.__enter__()

            for c in range(SC):
                nc.sync.dma_start(
                    x_sb[:, c, :],
                    x_d.rearrange("(c p) d -> p c d", p=P)[:, c, :],
                )

            xT = sg.tile([P, DC, S], BF, tag="xT")
            if "noxbar" not in ph:
                for st in range(NQ):
                    for dc in range(DC):
                        nc.sync.dma_start_transpose(
                            xT[:, dc, st * 512 : (st + 1) * 512],
                            xb_d[st * 512 : (st + 1) * 512, dc * P : (dc + 1) * P],
                        )
            else:
                nc.vector.memset(xT, 0.125)
            qkvT = sg.tile([P, 2, S], BF, tag="qkvT")  # j-tile: 0=Q^T 1=K^T
            vT_f32 = sg.tile([P, S], FP, tag="vT_f32")
            vt = [sg.tile([P, SC, 80], BF, tag=f"vt{h}", name=f"vt{h}") for h in range(H)]
            attnT = [sg.tile([DH, S], BF, tag=f"attnT{h}", name=f"attnT{h}") for h in range(H)]
            # denominator staging: lives on partition DH (=64), one S-wide
            # span per head (DVE copies are lane-aligned, so the psum row at
            # partition 64 can only land on SBUF partition 64)
            stage = sg.tile([P, H * S], FP, tag="stage")
            r_sb = sg.tile([P, H * SC], FP, tag="r_sb")

            # ---- phase 1+2: qkv projection, V prep ----
            if "pre" not in ph:
                nc.vector.memset(qkvT, 0.125)
                for h in range(H):
                    nc.vector.memset(vt[h], 0.125)
            with tc.tile_pool(name="ps_pre", bufs=2, space="PSUM") as pre:
                if "pre" not in ph:
                    pre_range = []
                else:
                    pre_range = [1, 0, 2]
                for jt in pre_range:
                    for st in range(NQ):
                        pq = pre.tile([P, 512], FP, tag="mm")
                        for dc in range(DC):
                            nc.tensor.matmul(
                                pq,
                                lhsT=w_sb[:, dc, jt * P : (jt + 1) * P],
                                rhs=xT[:, dc, st * 512 : (st + 1) * 512],
                                start=(dc == 0),
                                stop=(dc == DC - 1),
                            )
                        if jt == 2:
                            dst = vT_f32[:, st * 512 : (st + 1) * 512]
                        else:
                            dst = qkvT[:, jt, st * 512 : (st + 1) * 512]
                        if bias_sb is not None:
                            nc.vector.tensor_scalar_add(dst, pq, bias_sb[:, jt : jt + 1])
                        else:
                            nc.vector.tensor_copy(dst, pq)

                # Vtil: V with k on partitions, per head: [V(64 cols) | mask/ones col]
                for h in (range(H) if "pre" in ph else []):
                    if mask_sb is not None:
                        nc.vector.tensor_copy(vt[h][:, :, DH : DH + 1], mask_sb[:, :, None])
                    else:
                        nc.vector.memset(vt[h][:, :, DH : DH + 1], 1.0)
                for c in (range(SC) if "pre" in ph else []):
                    pt = pre.tile([P, P], FP, tag="tr")
                    nc.tensor.transpose(pt, vT_f32[:, c * P : (c + 1) * P], ident)
                    for h in range(H):
                        if mask_sb is not None:
                            nc.vector.tensor_scalar_mul(
                                vt[h][:, c, 0:DH],
                                pt[:, h * DH : (h + 1) * DH],
                                mask_sb[:, c : c + 1],
                            )
                        else:
                            nc.vector.tensor_copy(
                                vt[h][:, c, 0:DH], pt[:, h * DH : (h + 1) * DH]
                            )

            # ---- phase 4: attention core, one head at a time ----
            att_full = "att" in ph and "nopv" not in ph and "noexp" not in ph
            if not att_full:
                nc.vector.memset(stage[DH : DH + 1, :], 2048.0)
                for h in range(H):
                    nc.vector.memset(attnT[h], 0.01)
            with (
                tc.tile_pool(name="ps_pv", bufs=1, space="PSUM") as ppv,
                tc.tile_pool(name="ps_sc", bufs=2, space="PSUM") as psc,
                tc.tile_pool(name="expp", bufs=4) as expp,
            ):
                for h in (range(H) if "att" in ph else []):
                    hs = slice(h * DH, (h + 1) * DH)
                    pv = (
                        ppv.tile([P, S], FP, tag="pv", name="pv")
                        if att_full
                        else None
                    )
                    # software pipeline: emit pv(i-1) after scores(i) so the
                    # tensor engine never sits waiting on exp(i) (PE executes
                    # in program order; ACT runs exp(i) while PE does pv(i-1))
                    def emit_pv(pend):
                        pc, exs = pend
                        for phalf, pex in enumerate(exs):
                            for qq in range(2):
                                q0 = phalf * 1024 + qq * 512
                                nc.tensor.matmul(
                                    pv[0 : DH + 1, q0 : q0 + 512],
                                    lhsT=vt[h][:, pc, 0 : DH + 1],
                                    rhs=pex[:, qq * 512 : (qq + 1) * 512],
                                    start=(pc == 0),
                                    stop=(pc == SC - 1),
                                )

                    pending = []
                    for c in range(SC):
                        exs = []
                        for half in range(2):
                            sc_ps = psc.tile([P, 1024], FP, tag="sc")
                            for qq in range(2):
                                q0 = half * 1024 + qq * 512
                                nc.tensor.matmul(
                                    sc_ps[:, qq * 512 : (qq + 1) * 512],
                                    lhsT=qkvT[hs, 1, c * P : (c + 1) * P],
                                    rhs=qkvT[hs, 0, q0 : q0 + 512],
                                    start=True,
                                    stop=True,
                                )
                            if "noexp" in ph:
                                continue
                            ex = expp.tile([P, 1024], BF, tag="expS")
                            nc.scalar.activation(ex, sc_ps, AF.Exp, scale=SCALE)
                            exs.append(ex)
                        if "noexp" in ph or "nopv" in ph:
                            continue
                        pending.append((c, exs))
                        if len(pending) > 1:
                            emit_pv(pending.pop(0))
                    for pend in pending:
                        emit_pv(pend)
                    if att_full:
                        nc.scalar.copy(attnT[h][0:DH, :], pv[0:DH, :])
                        nc.scalar.copy(
                            stage[DH : DH + 1, h * S : (h + 1) * S], pv[DH : DH + 1, :]
                        )

            # ---- phase 6: transpose denominators, reciprocal ----
            with tc.tile_pool(name="ps_dn", bufs=1, space="PSUM") as pdn:
                dn = pdn.tile([P, H * SC], FP, tag="dn")
                for c in (range(SC) if "post" in ph else []):
                    for h in range(H):
                        nc.tensor.transpose(
                            dn[:, c * H + h : c * H + h + 1],
                            stage[DH : DH + 1, h * S + c * P : h * S + (c + 1) * P],
                            ident[DH : DH + 1, DH : DH + 1],
                        )
                if "post" in ph:
                    nc.vector.reciprocal(r_sb, dn)
                else:
                    nc.vector.memset(r_sb, 1.0 / 2048.0)

            # ---- phase 7: o-projection + combine + layernorm ----
            with (
                tc.tile_pool(name="ps_o", bufs=8, space="PSUM") as pso,
                tc.tile_pool(name="post", bufs=8) as post,
            ):
                y_t3 = y_d.rearrange("(c p) m -> p c m", p=P)
                for c in (range(SC) if "post" in ph else [0]):
                    po = []
                    for h in range(H):
                        po_t = pso.tile([P, DM], FP, tag="po", name=f"po{h}_{c}")
                        nc.tensor.matmul(
                            po_t,
                            lhsT=attnT[h][:, c * P : (c + 1) * P],
                            rhs=ow_sb[:, h, :],
                            start=True,
                            stop=True,
                        )
                        po.append(po_t)
                    t0 = post.tile([P, DM], FP, tag="t0")
                    nc.vector.scalar_tensor_tensor(
                        t0, po[0], r_sb[:, c * H : c * H + 1], x_sb[:, c, :],
                        op0=OP.mult, op1=OP.add,
                    )
                    y_t = post.tile([P, DM], FP, tag="y_t")
                    nc.vector.scalar_tensor_tensor(
                        y_t, po[1], r_sb[:, c * H + 1 : c * H + 2], t0,
                        op0=OP.mult, op1=OP.add,
                    )
                    st_t = post.tile([P, 6], FP, tag="st")
                    nc.vector.bn_stats(st_t, y_t)
                    mv = post.tile([P, 2], FP, tag="mv")
                    nc.vector.bn_aggr(mv, st_t)
                    sd = post.tile([P, 1], FP, tag="sd")
                    nc.scalar.activation(sd, mv[:, 1:2], AF.Sqrt, bias=eps_sb, scale=1.0)
                    rs = post.tile([P, 1], FP, tag="rs")
                    nc.vector.reciprocal(rs, sd)
                    o_t = post.tile([P, DM], FP, tag="o_t")
                    nc.vector.tensor_scalar(
                        o_t, y_t, scalar1=mv[:, 0:1], scalar2=rs,
                        op0=OP.subtract, op1=OP.mult,
                    )
                    if g_sb is not None and b_sb is not None:
                        nc.vector.tensor_mul(o_t, o_t, g_sb)
                        nc.vector.tensor_add(o_t, o_t, b_sb)
                    nc.sync.dma_start(y_t3[:, c, :], o_t)

            if rep_ctx is not None:
                rep_ctx.__exit__(None, None, None)

    nc.compile()
    return nc


_PROGRAM_CACHE: dict = {}


def _get_program(key):
    if key not in _PROGRAM_CACHE:
        _PROGRAM_CACHE[key] = _build(*key)
    return _PROGRAM_CACHE[key]


def kernel(x, attn_mask, qkv_w, qkv_b, o_w, ln_g, ln_b, **_ignored):
    x = np.ascontiguousarray(np.asarray(x, dtype=np.float32))
    attn_mask = np.asarray(attn_mask)
    qkv_w = np.ascontiguousarray(np.asarray(qkv_w, dtype=np.float32))
    qkv_b = np.asarray(qkv_b, dtype=np.float32)
    o_w = np.ascontiguousarray(np.asarray(o_w, dtype=np.float32))
    ln_g = np.asarray(ln_g, dtype=np.float32)
    ln_b = np.asarray(ln_b, dtype=np.float32)

    has_mask = not bool(attn_mask.all())
    has_bias = bool(np.any(qkv_b != 0.0))
    has_affine = bool(np.any(ln_g != 1.0) or np.any(ln_b != 0.0))

    nc = _get_program((has_mask, has_bias, has_affine))

    mask_f = attn_mask.astype(np.float32)
    in_maps = []
    for i in range(N_CORES):
        m = {
            "x": np.ascontiguousarray(x[i]),
            "x_bf": np.ascontiguousarray(x[i].astype(ml_dtypes.bfloat16)),
            "qkv_w_bf": qkv_w.astype(ml_dtypes.bfloat16),
            "o_w_bf": o_w.astype(ml_dtypes.bfloat16),
        }
        if has_mask:
            m["mask_f"] = np.ascontiguousarray(mask_f[i])
        if has_bias:
            m["qkv_b"] = qkv_b
        if has_affine:
            m["ln_g"] = ln_g
            m["ln_b"] = ln_b
        in_maps.append(m)

    trace = os.environ.get("KBENCH_TRACE", "0") == "1"
    kw = {}
    if trace:
        kw = {"trace": True, "trace_cores": [0]}
    res = run_bass_kernel_spmd(nc, in_maps, core_ids=list(range(N_CORES)), **kw)
    global LAST_RESULT
    LAST_RESULT = res
    return np.stack([res.results[i]["y"] for i in range(N_CORES)], axis=0)


LAST_RESULT = None



# revision 2
# speedup vs baseline: 1.0716x; 1.0716x over previous
"""Trainium2 Bass kernel v2 for 2-head MultiHeadAttn + residual + LayerNorm.

Problem shapes (hardcoded):
  x: [8, 2048, 384] f32   attn_mask: [8, 2048] bool   out: [8, 2048, 384] f32
  qkv_w: [384, 384], o_w: [128, 384], ln_g/ln_b: [384]

Sharding: data-parallel over batch, one batch element per NeuronCore (SPMD).

Design (per core), ACT(exp)-paced pipeline:
  - x shipped once as bf16; xT via DMA transposes on the scalar HWDGE queue,
    row tiles for the residual on the sync queue. The residual uses bf16 x
    (adds ~2e-3 rel err vs the 2e-2 budget, saves the 3.1MB fp32 load).
  - Q^T,K^T = (x@Wqk)^T via lhsT=W chunk, rhs=xT -> psum -> DVE cast bf16.
    Heads on partition halves: head0 = partitions 0..63, head1 = 64..127.
  - V computed directly in [s-part, d] orientation (lhsT=xT chunk, rhs=Wv)
    -> DVE cast to fp8e4 tiles vt{h} [128, 8 pairs, 2, 80]. The softmax
    denominator rides along as a ones column: col 64 for head0, col 65 for
    head1 (so the two heads' denominator rows land on psum partitions 64
    and 65 — lane-aligned for the DVE evacuation).
  - All of PSUM is ONE manually-slotted tile psall [128, 8, 512]:
    banks {2k,2k+1} (k=0..2) form a 3-deep ring of per-chunk score pairs,
    banks 6/7 hold the two heads' PV accumulators; o-proj and the
    denominator transposes borrow just-consumed score banks. Dependencies
    are AP-overlap tracked, so no pool ring is poisoned.
  - Attention in 4 q-passes of 512 columns. Per k-chunk c: scores^T for
    both heads (two matmuls, head h contraction on array rows h*64..) into
    the chunk's bank pair -> ONE ACT exp [128,1024] -> fp8 ring ex.
    exp(s*scale - 1): values ~0.37 sit in fp8e4 normal range; the -1 bias
    cancels between numerator and denominator. PV runs as fp8 DoubleRow
    matmuls (contraction 256) per chunk pair, emitted FOUR chunks late so
    the strict-FIFO PE queue never stalls on a fresh exp.
  - Post (per pass, interleaved into the next pass): denominators batch-
    transposed [2,128]->[128,2] -> reciprocal; o-proj per head into
    borrowed banks; y = x + po0*r0 + po1*r1; LayerNorm via bn_stats/
    bn_aggr with sqrt batched once per pass; DMA out.
"""

import os
import sys

import ml_dtypes
import numpy as np

for _p in ("/opt/trn_rl_repo", "/root/.axon_site/_ro/trn_rl_repo"):
    if os.path.isdir(_p) and _p not in sys.path:
        sys.path.insert(0, _p)

import concourse.bass as bass  # noqa: E402
import concourse.tile as tile  # noqa: E402
from concourse import bacc  # noqa: E402
from concourse import mybir  # noqa: E402
from concourse.bass_utils import run_bass_kernel_spmd  # noqa: E402
from concourse.masks import make_identity  # noqa: E402

FP = mybir.dt.float32
BF = mybir.dt.bfloat16
F8 = mybir.dt.float8e4
AF = mybir.ActivationFunctionType
OP = mybir.AluOpType
PM = mybir.MatmulPerfMode

B, S, DM = 8, 2048, 384
H, DH = 2, 64
INNER = H * DH  # 128
P = 128
SC = S // P  # 16 k-chunks of 128
DC = DM // P  # 3 model-dim chunks
NP = 4  # q passes
QT = S // NP  # 512 q columns per pass
RING = 6  # exp ring slots (pairs always land on (even, even+1))
LN_EPS = 1e-3
N_CORES = 8
SCALE = 1.0 / (DH**0.5)
EXP_BIAS = -1.0
PVLAG = 4  # chunks between exp production and PV consumption


def _build(has_mask: bool, has_bias: bool, has_affine: bool) -> bass.Bass:
    nc = bacc.Bacc(
        "TRN2", target_bir_lowering=False, debug=False, num_devices=N_CORES
    )

    xb_d = nc.dram_tensor("x_bf", [S, DM], BF, kind="ExternalInput")
    w_d = nc.dram_tensor("qkv_w_bf", [DM, 3 * INNER], BF, kind="ExternalInput")
    ow_d = nc.dram_tensor("o_w_bf", [INNER, DM], BF, kind="ExternalInput")
    mask_d = bias_d = g_d = b_d = None
    if has_mask:
        mask_d = nc.dram_tensor("mask_f", [S], FP, kind="ExternalInput")
    if has_bias:
        bias_d = nc.dram_tensor("qkv_b", [3 * INNER], FP, kind="ExternalInput")
    if has_affine:
        g_d = nc.dram_tensor("ln_g", [DM], FP, kind="ExternalInput")
        b_d = nc.dram_tensor("ln_b", [DM], FP, kind="ExternalInput")
    y_d = nc.dram_tensor("y", [S, DM], FP, kind="ExternalOutput")
    y_t3 = y_d.rearrange("(c p) m -> p c m", p=P)

    with tile.TileContext(nc) as tc:
        with (
            tc.tile_pool(name="singles", bufs=1) as sg,
            tc.tile_pool(name="allps", bufs=1, space="PSUM") as psp,
            tc.tile_pool(name="att", bufs=2) as asb,
            tc.tile_pool(name="ypool", bufs=5) as yp,
        ):
            psall = psp.tile([P, 8, 512], FP, tag="psall")

            ident = sg.tile([P, P], FP, tag="ident")
            make_identity(nc, ident)

            w_sb = sg.tile([P, DC, 3 * INNER], BF, tag="w_sb")
            nc.sync.dma_start(w_sb, w_d.rearrange("(dc dp) j -> dp dc j", dp=P))
            ow_sb = sg.tile([DH, H, DM], BF, tag="ow_sb")
            nc.sync.dma_start(ow_sb, ow_d.rearrange("(h d) m -> d h m", d=DH))
            xT = sg.tile([P, DC, S], BF, tag="xT")
            # x^T transposes split across the two HWDGE queues; the DMA
            # transpose wall (~18us) overlaps the folded-in qkv/V work below
            tq = [nc.sync, nc.scalar]
            for st in range(4):
                for dc in range(DC):
                    tq[(st * DC + dc) % 2].dma_start_transpose(
                        xT[:, dc, st * 512 : (st + 1) * 512],
                        xb_d[st * 512 : (st + 1) * 512, dc * P : (dc + 1) * P],
                    )
            x_sb = sg.tile([P, SC, DM], BF, tag="x_sb")
            x_rows = xb_d.rearrange("(c p) d -> p c d", p=P)
            for c in range(SC):
                nc.sync.dma_start(x_sb[:, c, :], x_rows[:, c, :])

            # preload the ln+exp combined ACT table set (act_info.json index
            # 6 = natural_log_exp_and_others) so the per-pass rstd =
            # exp(-0.5*ln(v+eps)) never swaps tables mid-exp-stream
            nc.scalar.add_instruction(
                mybir.InstLoadActFuncSet(
                    name=nc.get_next_instruction_name(),
                    ins=[],
                    outs=[],
                    act_func_set_id=6,
                )
            )
            eps_sb = sg.tile([P, 1], FP, tag="eps")
            nc.vector.memset(eps_sb, LN_EPS)
            ebias_sb = sg.tile([P, 1], FP, tag="ebias")
            nc.vector.memset(ebias_sb, EXP_BIAS)

            mask_sb = bias_sb = g_sb = b_sb = None
            if mask_d is not None:
                mask_sb = sg.tile([P, SC], FP, tag="mask_sb")
                nc.sync.dma_start(mask_sb, mask_d.rearrange("(c p) -> p c", p=P))
            if bias_d is not None:
                bias_sb = sg.tile([P, 3], FP, tag="bias_sb")
                nc.sync.dma_start(bias_sb, bias_d.rearrange("(jt p) -> p jt", p=P))
            if g_d is not None and b_d is not None:
                g_sb = sg.tile([P, DM], FP, tag="g_sb")
                b_sb = sg.tile([P, DM], FP, tag="b_sb")
                nc.gpsimd.dma_start(g_sb, g_d[None, :].to_broadcast((P, DM)))
                nc.gpsimd.dma_start(b_sb, b_d[None, :].to_broadcast((P, DM)))

            qkvT = sg.tile([P, 2, S], BF, tag="qkvT")  # 0=Q^T 1=K^T
            # fp8 V tiles per head; ones column at 64+h for the denominator
            vt = [
                sg.tile([P, SC // 2, 2, 80], F8, tag=f"vt{h}", name=f"vt{h}")
                for h in range(H)
            ]
            for h in range(H):
                nc.vector.memset(vt[h][:, :, :, DH:], 0.0)
                if mask_sb is None:
                    nc.vector.memset(vt[h][:, :, :, DH + h : DH + h + 1], 1.0)
            ex = sg.tile([P, RING, 1024], F8, tag="ex")  # exp ring

            def emit_qkv_jt(st, jt, bank):
                sl = slice(st * 512, (st + 1) * 512)
                pq = psall[:, bank, :]
                for dc in range(DC):
                    nc.tensor.matmul(
                        pq,
                        lhsT=w_sb[:, dc, jt * P : (jt + 1) * P],
                        rhs=xT[:, dc, sl],
                        start=(dc == 0),
                        stop=(dc == DC - 1),
                    )
                if bias_sb is not None:
                    nc.vector.tensor_scalar_add(
                        qkvT[:, jt, sl], pq, bias_sb[:, jt : jt + 1]
                    )
                else:
                    nc.vector.tensor_copy(qkvT[:, jt, sl], pq)

            def emit_vdirect(bank, col0, ci):
                """V chunk ci in [s-part, j] orientation -> fp8 vt tiles."""
                vd = psall[:, bank, col0 : col0 + INNER]
                for dc in range(DC):
                    nc.tensor.matmul(
                        vd,
                        lhsT=xT[:, dc, ci * P : (ci + 1) * P],
                        rhs=w_sb[:, dc, 2 * P : 3 * P],
                        start=(dc == 0),
                        stop=(dc == DC - 1),
                    )
                if bias_sb is not None:
                    nc.vector.tensor_scalar_add(vd, vd, bias_sb[:, 2:3])
                for h in range(H):
                    dst = vt[h][:, ci // 2, ci % 2, 0:DH]
                    src = vd[:, h * DH : (h + 1) * DH]
                    if mask_sb is not None:
                        nc.vector.tensor_scalar_mul(dst, src, mask_sb[:, ci : ci + 1])
                        nc.vector.tensor_copy(
                            vt[h][:, ci // 2, ci % 2, DH + h : DH + h + 1],
                            mask_sb[:, ci : ci + 1],
                        )
                    else:
                        nc.vector.tensor_copy(dst, src)

            def emit_pv(gc0):
                """PV DoubleRow pair for global chunks (gc0, gc0+1)."""
                cp = (gc0 % SC) // 2
                sl = gc0 % RING
                for h in range(H):
                    nc.tensor.matmul(
                        psall[0 : DH + 2, 6 + h, :],
                        lhsT=vt[h][:, cp, :, 0 : DH + 2],
                        rhs=ex[:, sl : sl + 2, h * 512 : (h + 1) * 512],
                        start=(cp == 0),
                        stop=(cp == SC // 2 - 1),
                        perf_mode=PM.DoubleRow,
                    )

            def emit_post_chunk(pp, j, state, borrow, use_act=False):
                attnT_p, r_p, mv8_p = state
                g = 4 * pp + j
                for h in range(H):
                    nc.tensor.matmul(
                        psall[:, borrow + h, 0:DM],
                        lhsT=attnT_p[:, h, j * P : (j + 1) * P],
                        rhs=ow_sb[:, h, :],
                        start=True,
                        stop=True,
                    )
                y_t = yp.tile([P, DM], FP, tag="y_t")
                if use_act:
                    # tail-only: ACT is idle there, so it applies the 1/denom
                    # scales while DVE just sums
                    a_t = asb.tile([P, DM], FP, tag="a_t")
                    nc.scalar.activation(
                        a_t, psall[:, borrow, 0:DM], AF.Copy,
                        scale=r_p[:, 2 * j : 2 * j + 1],
                    )
                    b_t = asb.tile([P, DM], FP, tag="b_t")
                    nc.scalar.activation(
                        b_t, psall[:, borrow + 1, 0:DM], AF.Copy,
                        scale=r_p[:, 2 * j + 1 : 2 * j + 2],
                    )
                    t0 = asb.tile([P, DM], FP, tag="t0")
                    nc.vector.tensor_add(t0, a_t, b_t)
                    nc.vector.tensor_add(y_t, t0, x_sb[:, g, :])
                else:
                    t0 = asb.tile([P, DM], FP, tag="t0")
                    nc.vector.scalar_tensor_tensor(
                        t0, psall[:, borrow, 0:DM], r_p[:, 2 * j : 2 * j + 1],
                        x_sb[:, g, :], op0=OP.mult, op1=OP.add,
                    )
                    nc.vector.scalar_tensor_tensor(
                        y_t, psall[:, borrow + 1, 0:DM],
                        r_p[:, 2 * j + 1 : 2 * j + 2],
                        t0, op0=OP.mult, op1=OP.add,
                    )
                st_t = asb.tile([P, 6], FP, tag="st")
                nc.vector.bn_stats(st_t, y_t)
                nc.vector.bn_aggr(mv8_p[:, 2 * j : 2 * j + 2], st_t)
                return y_t

            def emit_post_tail(pp, state, y_ts):
                _, _, mv8_p = state
                # rstd = exp(-0.5*ln(v+eps)): Ln and Exp share one ACT table
                # set, so no 1.3us table swaps around the softmax exp stream
                lnv = asb.tile([P, 8], FP, tag="lnv")
                nc.scalar.activation(lnv, mv8_p, AF.Ln, bias=eps_sb, scale=1.0)
                rs8 = asb.tile([P, 8], FP, tag="rs8")
                nc.scalar.activation(rs8, lnv, AF.Exp, scale=-0.5)
                for j in range(4):
                    g = 4 * pp + j
                    o_t = yp.tile([P, DM], FP, tag="o_t")
                    nc.vector.tensor_scalar(
                        o_t, y_ts[j],
                        scalar1=mv8_p[:, 2 * j : 2 * j + 1],
                        scalar2=rs8[:, 2 * j + 1 : 2 * j + 2],
                        op0=OP.subtract, op1=OP.mult,
                    )
                    if g_sb is not None and b_sb is not None:
                        nc.vector.tensor_mul(o_t, o_t, g_sb)
                        nc.vector.tensor_add(o_t, o_t, b_sb)
                    nc.sync.dma_start(y_t3[:, g, :], o_t)

            def emit_evac_copies():
                """Evacuate pv banks -> SBUF (attnT + stacked denom rows)."""
                attnT_t = asb.tile([DH, H, 512], BF, tag="attnT")
                stage = asb.tile([P, 512], FP, tag="stage")
                for h in range(H):
                    nc.vector.tensor_copy(attnT_t[:, h, :], psall[0:DH, 6 + h, :])
                # pv0 row 65 and pv1 row 64 are zero (vt pad), so adding the
                # two [64:66] windows stacks denom0/denom1 on rows 64/65
                nc.vector.tensor_copy(stage[DH : DH + 2, :], psall[DH : DH + 2, 6, :])
                nc.vector.tensor_add(
                    stage[DH : DH + 2, :],
                    stage[DH : DH + 2, :],
                    psall[DH : DH + 2, 7, :],
                )
                return attnT_t, stage

            def emit_dn(attnT_t, stage, dnb):
                """Transpose denominators -> reciprocal -> pass state."""
                for j in range(4):
                    nc.tensor.transpose(
                        psall[:, dnb, 2 * j : 2 * j + 2],
                        stage[DH : DH + 2, j * P : (j + 1) * P],
                        ident[DH : DH + 2, DH : DH + 2],
                    )
                r_sb = asb.tile([P, 8], FP, tag="r_sb")
                nc.vector.reciprocal(r_sb, psall[:, dnb, 0:8])
                mv8 = asb.tile([P, 8], FP, tag="mv8")
                return (attnT_t, r_sb, mv8)

            # ---- startup: qkv st0 + V chunks 0..3; the rest of qkv and
            # V-direct is folded into pass 0's chunk slots below ----
            emit_qkv_jt(0, 1, 4)
            emit_qkv_jt(0, 0, 5)
            for ci in range(4):
                emit_vdirect(6 + ci % 2, 0, ci)

            # ---- attention passes ----
            # per pass: 16 score pairs + exps stream; the previous pass's
            # trailing PV pairs, pv-bank evacuation and post-processing are
            # all folded into this pass's chunk slots so neither engine
            # stalls at the boundary.
            # In passes 1-3 exps run as [128,2048] duos whenever the two
            # chunks' bank pairs are contiguous (gc%6 in {0,4} defers to a
            # duo at gc+1); anything that borrows the (gc-1) bank pair is
            # scheduled only at chunks where exp(gc-1) was NOT deferred.
            DN_AT = {1: 1, 2: 1, 3: 1}
            POST_P = {
                1: {3: 0, 6: 1, 10: 2, 14: 3},
                2: {3: 0, 6: 1, 10: 2, 14: 3},
                3: {3: 0, 6: 1, 10: 2, 14: 3},
            }
            TAIL_AT = {1: 15, 2: 15, 3: 15}
            prev = None  # (attnT, r_sb, mv8) of previous pass
            evac_sb = None
            y_ts_prev = [None] * 4
            for p in range(NP):
                q0 = p * QT
                for c in range(SC):
                    gc = p * SC + c
                    pair = 2 * (gc % 3)
                    for h in range(H):
                        hs = slice(h * DH, (h + 1) * DH)
                        nc.tensor.matmul(
                            psall[:, pair + h, :],
                            lhsT=qkvT[hs, 1, c * P : (c + 1) * P],
                            rhs=qkvT[hs, 0, q0 : q0 + 512],
                            start=True,
                            stop=True,
                        )
                    nc.scalar.activation(
                        ex[:, gc % RING, :], psall[:, pair : pair + 2, :],
                        AF.Exp, scale=SCALE, bias=ebias_sb,
                    )
                    if p == 0:
                        # fold the remaining qkv spans and V chunks into
                        # pass 0, borrowing just-consumed score banks
                        if c in (0, 5, 9):
                            emit_qkv_jt(c // 4 + 1, 1, 2 * ((gc - 1) % 3))
                        elif c in (1, 6, 10):
                            emit_qkv_jt(c // 4 + 1, 0, 2 * ((gc - 1) % 3))
                        if 1 <= c <= 12:
                            emit_vdirect(2 * ((gc - 1) % 3) + 1, 256, c + 3)
                    if p > 0:
                        if c == 0:
                            emit_pv(gc - 4)
                        elif c == 1:
                            emit_pv(gc - 3)
                            evac_sb = emit_evac_copies()
                        if c == DN_AT[p]:
                            prev = emit_dn(*evac_sb, 2 * ((gc - 1) % 3))
                        elif c in POST_P[p]:
                            y_ts_prev[POST_P[p][c]] = emit_post_chunk(
                                p - 1, POST_P[p][c], prev, 2 * ((gc - 1) % 3)
                            )
                        elif c == TAIL_AT[p]:
                            emit_post_tail(p - 1, prev, y_ts_prev)
                    if c % 2 == 0 and c >= PVLAG:
                        emit_pv(gc - PVLAG)

            # last pass trailing work
            gce = NP * SC
            emit_pv(gce - 4)
            emit_pv(gce - 2)
            prev = emit_dn(*emit_evac_copies(), 2 * ((gce - 1) % 3))
            for j in range(4):
                y_ts_prev[j] = emit_post_chunk(
                    NP - 1, j, prev, 2 * ((gce - 1 + 2 * j) % 3), use_act=True
                )
            emit_post_tail(NP - 1, prev, y_ts_prev)

    nc.compile()
    return nc


_PROGRAM_CACHE: dict = {}


def _get_program(key):
    if key not in _PROGRAM_CACHE:
        _PROGRAM_CACHE[key] = _build(*key)
    return _PROGRAM_CACHE[key]


def kernel(x, attn_mask, qkv_w, qkv_b, o_w, ln_g, ln_b, **_ignored):
    x = np.ascontiguousarray(np.asarray(x, dtype=np.float32))
    attn_mask = np.asarray(attn_mask)
    qkv_w = np.ascontiguousarray(np.asarray(qkv_w, dtype=np.float32))
    qkv_b = np.asarray(qkv_b, dtype=np.float32)
    o_w = np.ascontiguousarray(np.asarray(o_w, dtype=np.float32))
    ln_g = np.asarray(ln_g, dtype=np.float32)
    ln_b = np.asarray(ln_b, dtype=np.float32)

    has_mask = not bool(attn_mask.all())
    has_bias = bool(np.any(qkv_b != 0.0))
    has_affine = bool(np.any(ln_g != 1.0) or np.any(ln_b != 0.0))

    nc = _get_program((has_mask, has_bias, has_affine))

    mask_f = attn_mask.astype(np.float32)
    in_maps = []
    for i in range(N_CORES):
        m = {
            "x_bf": np.ascontiguousarray(x[i].astype(ml_dtypes.bfloat16)),
            "qkv_w_bf": qkv_w.astype(ml_dtypes.bfloat16),
            "o_w_bf": o_w.astype(ml_dtypes.bfloat16),
        }
        if has_mask:
            m["mask_f"] = np.ascontiguousarray(mask_f[i])
        if has_bias:
            m["qkv_b"] = qkv_b
        if has_affine:
            m["ln_g"] = ln_g
            m["ln_b"] = ln_b
        in_maps.append(m)

    trace = os.environ.get("KBENCH_TRACE", "0") == "1"
    kw = {}
    if trace:
        kw = {"trace": True, "trace_cores": [0]}
    res = run_bass_kernel_spmd(nc, in_maps, core_ids=list(range(N_CORES)), **kw)
    global LAST_RESULT
    LAST_RESULT = res
    return np.stack([res.results[i]["y"] for i in range(N_CORES)], axis=0)


LAST_RESULT = None
